# revision 5
# baseline (speedup 1.0000x reference)
"""MoE FFN (8 experts, top-2) on 8 TRN2 NeuronCores — d_ff-sliced tensor
parallel.

Strategy:
  - Host: compute gate logits (fp64), top-2 selection + softmax weights
    exactly as the reference; build the expert-sorted column stream of all
    T*2 (token, expert) pairs (identical for every core).
  - Device (SPMD): every core processes ALL columns but only a 512-wide
    slice of d_ff: h_k = relu(W1[e][:, slice_k]^T x + b1[slice_k]),
    y_k = W2[e][slice_k, :]^T h_k.  Per-core compute is exactly
    (2*T*1024*512*2*2) flops regardless of routing balance, and the tile
    schedule (derived from the expert counts) is identical on every core,
    so one SPMD program serves all cores; only the weight-slice contents
    differ per core.
  - Host: y(pair) = sum_k y_k(pair) + b2[e]; out[token] += gate_w * y.

Shapes (hardcoded from the problem):
  x: [4, 1024, 1024] f32, Wg: [1024, 8], bg: [8],
  W1: [8, 1024, 4096], b1: [8, 4096], W2: [8, 4096, 1024], b2: [8, 1024]
"""

import math

import ml_dtypes
import numpy as np

MODEL_DIM = 1024
DIM_FF = 4096
NUM_EXPERTS = 8
TOP_K = 2
N_CORES = 8
F_SLICE = DIM_FF // N_CORES          # 512
FMS = F_SLICE // 128                 # 4 fm blocks per core
DKS = MODEL_DIM // 128               # 8
DMS = MODEL_DIM // 128               # 8
MAX_TW = 512                         # PSUM bank limit (fp32 cols)

BF16 = ml_dtypes.bfloat16

_NC_CACHE: dict[tuple, object] = {}


def _tile_widths(c):
    """Split a padded expert count into near-equal tile widths (mult of 4,
    each <= MAX_TW)."""
    C = ((c + 3) // 4) * 4
    n = max(1, math.ceil(C / MAX_TW))
    base = C // n
    base -= base % 4
    widths = [base] * n
    extra = C - base * n  # multiple of 4
    i = 0
    while extra > 0:
        add = min(4, extra)
        widths[i % n] += add
        extra -= add
        i += 1
    return widths


def _make_schedule(counts):
    """-> (tiles, Ctot): tiles = tuple of (expert, width, col0)."""
    tiles = []
    col0 = 0
    for e in range(NUM_EXPERTS):
        if counts[e] == 0:
            continue
        for w in _tile_widths(counts[e]):
            tiles.append((e, w, col0))
            col0 += w
    return tuple(tiles), col0


def _build_tp_nc(tiles, Ctot):
    """Build + compile the SPMD Bass program for the tile schedule."""
    import concourse.mybir as mybir
    import concourse.tile as tile
    from concourse import bacc

    used_experts = sorted({e for e, _, _ in tiles})

    nc = bacc.Bacc("TRN2", target_bir_lowering=False)
    w1_d = nc.dram_tensor("w1", [128, NUM_EXPERTS * FMS * DKS * 128],
                          mybir.dt.bfloat16, kind="ExternalInput")
    w2_d = nc.dram_tensor("w2", [128, NUM_EXPERTS * DMS * FMS * 128],
                          mybir.dt.bfloat16, kind="ExternalInput")
    b1_d = nc.dram_tensor("b1", [128, NUM_EXPERTS * FMS], mybir.dt.float32,
                          kind="ExternalInput")
    xs_d = nc.dram_tensor("xs", [128, DKS * Ctot], mybir.dt.bfloat16,
                          kind="ExternalInput")
    y_d = nc.dram_tensor("y", [MODEL_DIM, Ctot], mybir.dt.bfloat16,
                         kind="ExternalOutput")

    XS_PRE = 6   # xs tiles pre-issued before the weight stream
    XS_DEPTH = 3  # in-loop prefetch distance

    with tile.TileContext(nc) as tc:
        with (
            tc.tile_pool(name="pers", bufs=1) as pers,
            tc.tile_pool(name="xsp", bufs=XS_PRE) as xsp,
            tc.tile_pool(name="hp", bufs=2) as hp,
            tc.tile_pool(name="yp", bufs=4) as yp,
            tc.tile_pool(name="psp", bufs=8, space="PSUM") as psp,
        ):
            # Warmup: dummy matmuls keep PE busy (and start the HAM busy
            # window) while the first DMAs land.
            warm_sb = pers.tile([128, 512], mybir.dt.bfloat16,
                                tag="warm", name="warm_sb")
            nc.vector.memset(warm_sb, 0)
            warm_ps = psp.tile([128, 512], mybir.dt.float32, tag="ps",
                               name="warm_ps")
            N_WARM = 8
            for i in range(N_WARM):
                nc.tensor.matmul(
                    warm_ps, lhsT=warm_sb[:, :128], rhs=warm_sb,
                    start=(i == 0), stop=(i == N_WARM - 1),
                )

            b1t = pers.tile([128, NUM_EXPERTS * FMS], mybir.dt.float32,
                            tag="b1", name="b1t")
            nc.sync.dma_start(b1t, b1_d[:, :])

            e0 = tiles[0][0]
            w1t, w2t = {}, {}
            for e in used_experts:
                w1t[e] = pers.tile([128, FMS * DKS * 128], mybir.dt.bfloat16,
                                   tag=f"w1_{e}", name=f"w1t_{e}")
                w2t[e] = pers.tile([128, DMS * FMS * 128], mybir.dt.bfloat16,
                                   tag=f"w2_{e}", name=f"w2t_{e}")

            xst = {}

            def issue_xs(t):
                e, w, col0 = tiles[t]
                xt = xsp.tile([128, DKS * w], mybir.dt.bfloat16,
                              tag="xs", name=f"xst_{t}")
                nc.sync.dma_start(
                    xt, xs_d[:, DKS * col0:DKS * col0 + DKS * w])
                xst[t] = xt

            # DMA issue order: first fm block of expert-0 W1, first xs tile,
            # rest of expert-0 weights, a couple more xs tiles, then the
            # remaining experts' weights interleaved with xs tiles.
            W1E0_HEAD = DKS * 128  # one fm block = [128, 1024]
            nc.sync.dma_start(w1t[e0][:, :W1E0_HEAD],
                              w1_d[:, e0 * FMS * DKS * 128:
                                   e0 * FMS * DKS * 128 + W1E0_HEAD])
            issue_xs(0)
            nc.sync.dma_start(w1t[e0][:, W1E0_HEAD:],
                              w1_d[:, e0 * FMS * DKS * 128 + W1E0_HEAD:
                                   (e0 + 1) * FMS * DKS * 128])
            nc.sync.dma_start(w2t[e0], w2_d[:, e0 * DMS * FMS * 128:
                                            (e0 + 1) * DMS * FMS * 128])
            n_xs_issued = 1
            for t in range(1, min(3, len(tiles))):
                issue_xs(t)
                n_xs_issued += 1
            for e in used_experts:
                if e == e0:
                    continue
                nc.sync.dma_start(w1t[e], w1_d[:, e * FMS * DKS * 128:
                                               (e + 1) * FMS * DKS * 128])
                nc.sync.dma_start(w2t[e], w2_d[:, e * DMS * FMS * 128:
                                               (e + 1) * DMS * FMS * 128])
                if n_xs_issued < min(XS_PRE, len(tiles)):
                    issue_xs(n_xs_issued)
                    n_xs_issued += 1
            while n_xs_issued < min(XS_PRE, len(tiles)):
                issue_xs(n_xs_issued)
                n_xs_issued += 1

            for t, (e, w, col0) in enumerate(tiles):
                nxt = t + XS_DEPTH
                if XS_PRE <= nxt < len(tiles):
                    issue_xs(nxt)

                # Phase A: h[fm] = relu(W1slice^T x + b1slice)
                hts = []
                for fm in range(FMS):
                    ps = psp.tile([128, w], mybir.dt.float32, tag="ps",
                                  name=f"ps1_{fm}_{t}")
                    for dk in range(DKS):
                        lo = (fm * DKS + dk) * 128
                        nc.tensor.matmul(
                            ps,
                            lhsT=w1t[e][:, lo:lo + 128],
                            rhs=xst[t][:, dk * w:(dk + 1) * w],
                            start=(dk == 0),
                            stop=(dk == DKS - 1),
                        )
                    ht = hp.tile([128, w], mybir.dt.bfloat16,
                                 tag=f"h_{fm}", name=f"ht_{fm}_{t}")
                    nc.vector.tensor_scalar(
                        out=ht, in0=ps,
                        scalar1=b1t[:, e * FMS + fm:e * FMS + fm + 1],
                        scalar2=0.0,
                        op0=mybir.AluOpType.add, op1=mybir.AluOpType.max,
                    )
                    hts.append(ht)

                # Phase B: y_partial[dm] = W2slice^T h
                for dm in range(DMS):
                    ps = psp.tile([128, w], mybir.dt.float32, tag="ps",
                                  name=f"ps2_{dm}_{t}")
                    for fk in range(FMS):
                        lo = (dm * FMS + fk) * 128
                        nc.tensor.matmul(
                            ps,
                            lhsT=w2t[e][:, lo:lo + 128],
                            rhs=hts[fk],
                            start=(fk == 0),
                            stop=(fk == FMS - 1),
                        )
                    yo = yp.tile([128, w], mybir.dt.bfloat16, tag="yo",
                                 name=f"yo_{dm}_{t}")
                    nc.vector.tensor_copy(yo, ps)
                    nc.sync.dma_start(
                        y_d[dm * 128:(dm + 1) * 128, col0:col0 + w], yo
                    )

    nc.compile()
    return nc


def _route_host(x, Wg, bg):
    """Reference-exact gate: fp64 logits, top-2 (ties -> lower index), softmax."""
    T = x.shape[0]
    logits = x.astype(np.float64) @ Wg.astype(np.float64) + bg.astype(np.float64)
    order = np.argsort(-logits, axis=1, kind="stable")[:, :TOP_K]  # [T, 2]
    vals = np.take_along_axis(logits, order, axis=1)
    vmax = vals.max(axis=1, keepdims=True)
    ev = np.exp(vals - vmax)
    w = (ev / ev.sum(axis=1, keepdims=True)).astype(np.float32)  # [T, 2]
    tok = np.repeat(np.arange(T), TOP_K)
    exp = order.ravel()
    wgt = w.ravel()
    tok_e, w_e = [], []
    for e in range(NUM_EXPERTS):
        m = exp == e
        tok_e.append(tok[m])
        w_e.append(wgt[m])
    return tok_e, w_e


def _pack_xs(xt, tok_e, tiles, Ctot):
    """Build the packed x stream [128, 8*Ctot] bf16 (identical per core).

    Column layout: expert-sorted pair stream, tile-major; within tile t of
    width w, dk block at cols [8*col0 + dk*w, ...)."""
    xT = np.zeros((MODEL_DIM, Ctot), dtype=BF16)
    col = 0
    seg = {}
    for e in range(NUM_EXPERTS):
        cnt = len(tok_e[e])
        if cnt == 0:
            continue
        xT[:, col:col + cnt] = xt[tok_e[e]].astype(BF16).T
        seg[e] = (col, cnt)
        col += sum(_tile_widths(cnt))
    out = np.empty((128, DKS * Ctot), dtype=BF16)
    for e, w, col0 in tiles:
        for dk in range(DKS):
            out[:, DKS * col0 + dk * w: DKS * col0 + (dk + 1) * w] = \
                xT[dk * 128:(dk + 1) * 128, col0:col0 + w]
    return np.ascontiguousarray(out), seg


def _pack_w1_slice(W1, k):
    """W1 [8, 1024, 4096] f32, core k -> [128, 8*4096] bf16.

    Block (e, fm, dk) at col e*4096 + (fm*8+dk)*128 =
    W1[e, dk*128:(dk+1)*128, k*512+fm*128 : +128]."""
    sl = W1[:, :, k * F_SLICE:(k + 1) * F_SLICE].astype(BF16)  # [8,1024,512]
    # [e, dk, 128, fm, 128] -> order (e, fm, dk)
    blk = sl.reshape(NUM_EXPERTS, DKS, 128, FMS, 128)
    out = blk.transpose(2, 0, 3, 1, 4).reshape(128, -1)
    return np.ascontiguousarray(out)


def _pack_w2_slice(W2, k):
    """W2 [8, 4096, 1024] f32, core k -> [128, 8*4096] bf16.

    Block (e, dm, fk) at col e*4096 + (dm*4+fk)*128 =
    W2[e, k*512+fk*128 : +128, dm*128:(dm+1)*128]."""
    sl = W2[:, k * F_SLICE:(k + 1) * F_SLICE, :].astype(BF16)  # [8,512,1024]
    blk = sl.reshape(NUM_EXPERTS, FMS, 128, DMS, 128)
    out = blk.transpose(2, 0, 3, 1, 4).reshape(128, -1)
    return np.ascontiguousarray(out)


def _pack_b1_slice(b1, k):
    """b1 [8, 4096] -> [128, 32] f32; col e*4+fm = b1[e, k*512+fm*128:+128]."""
    sl = b1[:, k * F_SLICE:(k + 1) * F_SLICE].astype(np.float32)  # [8, 512]
    out = sl.reshape(NUM_EXPERTS * FMS, 128).T
    return np.ascontiguousarray(out)


def _make_in_maps(xt, W1, b1, W2, tok_e, tiles, Ctot):
    xs_packed, seg = _pack_xs(xt, tok_e, tiles, Ctot)
    in_maps = []
    for k in range(N_CORES):
        in_maps.append({
            "w1": _pack_w1_slice(W1, k),
            "w2": _pack_w2_slice(W2, k),
            "b1": _pack_b1_slice(b1, k),
            "xs": xs_packed,
        })
    return in_maps, seg


def kernel(x, Wg, bg, W1, b1, W2, b2):
    from concourse.bass_utils import run_bass_kernel_spmd

    B, S, d = x.shape
    T = B * S
    xt = x.reshape(T, d)

    tok_e, w_e = _route_host(xt, Wg, bg)
    counts = [len(t) for t in tok_e]
    tiles, Ctot = _make_schedule(counts)

    if tiles not in _NC_CACHE:
        _NC_CACHE[tiles] = _build_tp_nc(tiles, Ctot)
    nc = _NC_CACHE[tiles]

    in_maps, seg = _make_in_maps(xt, W1, b1, W2, tok_e, tiles, Ctot)

    res = run_bass_kernel_spmd(nc, in_maps, core_ids=list(range(N_CORES)))

    # Combine: y(pair) = sum_k y_k(pair) + b2[e]; out[tok] += gate_w * y
    Y = res.results[0]["y"].astype(np.float32)
    for k in range(1, N_CORES):
        Y += res.results[k]["y"].astype(np.float32)
    out = np.zeros((T, d), dtype=np.float32)
    for e in range(NUM_EXPERTS):
        if e not in seg:
            continue
        col0, cnt = seg[e]
        ye = Y[:, col0:col0 + cnt].T + b2[e][None, :].astype(np.float32)
        out[tok_e[e]] += ye * w_e[e][:, None]
    return out.reshape(B, S, d)


# revision 8
# speedup vs baseline: 1.2581x; 1.2581x over previous
"""MoE FFN (8 experts, top-2) on 8 TRN2 NeuronCores — d_ff-sliced tensor
parallel.

Strategy:
  - Host: compute gate logits (fp64), top-2 selection + softmax weights
    exactly as the reference; build the expert-sorted column stream of all
    T*2 (token, expert) pairs (identical for every core).
  - Device (SPMD): every core processes ALL columns but only a 512-wide
    slice of d_ff: h_k = relu(W1[e][:, slice_k]^T x + b1[slice_k]),
    y_k = W2[e][slice_k, :]^T h_k.  Per-core compute is exactly
    (2*T*1024*512*2*2) flops regardless of routing balance, and the tile
    schedule (derived from the expert counts) is identical on every core,
    so one SPMD program serves all cores; only the weight-slice contents
    differ per core.
  - Host: y(pair) = sum_k y_k(pair) + b2[e]; out[token] += gate_w * y.

Shapes (hardcoded from the problem):
  x: [4, 1024, 1024] f32, Wg: [1024, 8], bg: [8],
  W1: [8, 1024, 4096], b1: [8, 4096], W2: [8, 4096, 1024], b2: [8, 1024]
"""

import math

import ml_dtypes
import numpy as np

MODEL_DIM = 1024
DIM_FF = 4096
NUM_EXPERTS = 8
TOP_K = 2
N_CORES = 8
F_SLICE = DIM_FF // N_CORES          # 512
FMS = F_SLICE // 128                 # 4 fm blocks per core
DKS = MODEL_DIM // 128               # 8
DMS = MODEL_DIM // 128               # 8
MAX_TW = 512                         # PSUM bank limit (fp32 cols)

BF16 = ml_dtypes.bfloat16

_NC_CACHE: dict[tuple, object] = {}


def _tile_widths(c):
    """Split a padded expert count into near-equal tile widths (mult of 4,
    each <= MAX_TW)."""
    C = ((c + 3) // 4) * 4
    n = max(1, math.ceil(C / MAX_TW))
    base = C // n
    base -= base % 4
    widths = [base] * n
    extra = C - base * n  # multiple of 4
    i = 0
    while extra > 0:
        add = min(4, extra)
        widths[i % n] += add
        extra -= add
        i += 1
    return widths


def _make_schedule(counts):
    """-> (tiles, Ctot): tiles = tuple of (expert, width, col0)."""
    tiles = []
    col0 = 0
    for e in range(NUM_EXPERTS):
        if counts[e] == 0:
            continue
        for w in _tile_widths(counts[e]):
            tiles.append((e, w, col0))
            col0 += w
    return tuple(tiles), col0


def _build_tp_nc(tiles, Ctot):
    """Build + compile the SPMD Bass program for the tile schedule.

    DMA discipline: keep the ring shallow. Pre-issue only b1, xs[0..3] and
    expert-0's weights (fm/dm-chunked so compute can start early); issue
    each subsequent expert's weights at the previous expert's first tile;
    prefetch xs 4 tiles ahead; stage y in [128, 4w] half-tiles so output
    rows are >=2.7KB.
    """
    import concourse.mybir as mybir
    import concourse.tile as tile
    from concourse import bacc

    used_experts = sorted({e for e, _, _ in tiles})
    first_tile_of = {}
    for t, (e, w, c0) in enumerate(tiles):
        if e not in first_tile_of:
            first_tile_of[e] = t
    expert_idx = {e: i for i, e in enumerate(used_experts)}

    nc = bacc.Bacc("TRN2", target_bir_lowering=False)
    w1_d = nc.dram_tensor("w1", [128, NUM_EXPERTS * FMS * DKS * 128],
                          mybir.dt.bfloat16, kind="ExternalInput")
    w2_d = nc.dram_tensor("w2", [128, NUM_EXPERTS * DMS * FMS * 128],
                          mybir.dt.bfloat16, kind="ExternalInput")
    b1_d = nc.dram_tensor("b1", [128, NUM_EXPERTS * FMS], mybir.dt.float32,
                          kind="ExternalInput")
    xs_d = nc.dram_tensor("xs", [128, DKS * Ctot], mybir.dt.bfloat16,
                          kind="ExternalInput")
    y_d = nc.dram_tensor("y", [128, DKS * Ctot], mybir.dt.bfloat16,
                         kind="ExternalOutput")

    XS_PRE = 4    # xs tiles pre-issued
    XS_DEPTH = 4  # in-loop prefetch distance

    with tile.TileContext(nc) as tc:
        with (
            tc.tile_pool(name="pers", bufs=1) as pers,
            tc.tile_pool(name="xsp", bufs=6) as xsp,
            tc.tile_pool(name="hp", bufs=2) as hp,
            tc.tile_pool(name="yp", bufs=2) as yp,
            tc.tile_pool(name="psp", bufs=8, space="PSUM") as psp,
        ):
            # Warmup: dummy matmuls keep PE busy (and start the HAM busy
            # window) while the first DMAs land.
            warm_sb = pers.tile([128, 512], mybir.dt.bfloat16,
                                tag="warm", name="warm_sb")
            nc.vector.memset(warm_sb, 0)
            warm_ps = psp.tile([128, 512], mybir.dt.float32, tag="ps",
                               name="warm_ps")
            N_WARM = 10
            for i in range(N_WARM):
                nc.tensor.matmul(
                    warm_ps, lhsT=warm_sb[:, :128], rhs=warm_sb,
                    start=(i == 0), stop=(i == N_WARM - 1),
                )

            b1t = pers.tile([128, NUM_EXPERTS * FMS], mybir.dt.float32,
                            tag="b1", name="b1t")
            nc.sync.dma_start(b1t, b1_d[:, :])

            w1t, w2t = {}, {}
            for e in used_experts:
                w1t[e] = pers.tile([128, FMS * DKS * 128], mybir.dt.bfloat16,
                                   tag=f"w1_{e}", name=f"w1t_{e}")
                w2t[e] = pers.tile([128, DMS * FMS * 128], mybir.dt.bfloat16,
                                   tag=f"w2_{e}", name=f"w2t_{e}")

            def issue_w(e, chunked):
                w1o = e * FMS * DKS * 128
                w2o = e * DMS * FMS * 128
                if chunked:
                    for fm in range(FMS):
                        lo = fm * DKS * 128
                        nc.sync.dma_start(w1t[e][:, lo:lo + DKS * 128],
                                          w1_d[:, w1o + lo:w1o + lo + DKS * 128])
                    half = DMS * FMS * 128 // 2
                    nc.sync.dma_start(w2t[e][:, :half],
                                      w2_d[:, w2o:w2o + half])
                    nc.sync.dma_start(w2t[e][:, half:],
                                      w2_d[:, w2o + half:w2o + 2 * half])
                else:
                    nc.sync.dma_start(w1t[e], w1_d[:, w1o:w1o + FMS * DKS * 128])
                    nc.sync.dma_start(w2t[e], w2_d[:, w2o:w2o + DMS * FMS * 128])

            xst = {}

            def issue_xs(t):
                e, w, col0 = tiles[t]
                xt = xsp.tile([128, DKS * w], mybir.dt.bfloat16,
                              tag="xs", name=f"xst_{t}")
                nc.sync.dma_start(
                    xt, xs_d[:, DKS * col0:DKS * col0 + DKS * w])
                xst[t] = xt

            issue_xs(0)
            issue_w(used_experts[0], chunked=True)
            for t in range(1, min(XS_PRE, len(tiles))):
                issue_xs(t)

            for t, (e, w, col0) in enumerate(tiles):
                nxt = t + XS_DEPTH
                if XS_PRE <= nxt < len(tiles):
                    issue_xs(nxt)
                if first_tile_of[e] == t:
                    i = expert_idx[e]
                    if i + 1 < len(used_experts):
                        issue_w(used_experts[i + 1], chunked=False)

                # Phase A: h[fm] = relu(W1slice^T x + b1slice)
                hts = []
                for fm in range(FMS):
                    ps = psp.tile([128, w], mybir.dt.float32, tag="ps",
                                  name=f"ps1_{fm}_{t}")
                    for dk in range(DKS):
                        lo = (fm * DKS + dk) * 128
                        nc.tensor.matmul(
                            ps,
                            lhsT=w1t[e][:, lo:lo + 128],
                            rhs=xst[t][:, dk * w:(dk + 1) * w],
                            start=(dk == 0),
                            stop=(dk == DKS - 1),
                        )
                    ht = hp.tile([128, w], mybir.dt.bfloat16,
                                 tag=f"h_{fm}", name=f"ht_{fm}_{t}")
                    nc.vector.tensor_scalar(
                        out=ht, in0=ps,
                        scalar1=b1t[:, e * FMS + fm:e * FMS + fm + 1],
                        scalar2=0.0,
                        op0=mybir.AluOpType.add, op1=mybir.AluOpType.max,
                    )
                    hts.append(ht)

                # Phase B: y_partial[dm] = W2slice^T h, staged in two
                # [128, 4w] halves so output DMA rows are 4w*2B.
                stages = [
                    yp.tile([128, 4 * w], mybir.dt.bfloat16, tag="ya",
                            name=f"ya_{t}"),
                    yp.tile([128, 4 * w], mybir.dt.bfloat16, tag="yb",
                            name=f"yb_{t}"),
                ]
                for dm in range(DMS):
                    ps = psp.tile([128, w], mybir.dt.float32, tag="ps",
                                  name=f"ps2_{dm}_{t}")
                    for fk in range(FMS):
                        lo = (dm * FMS + fk) * 128
                        nc.tensor.matmul(
                            ps,
                            lhsT=w2t[e][:, lo:lo + 128],
                            rhs=hts[fk],
                            start=(fk == 0),
                            stop=(fk == FMS - 1),
                        )
                    st = stages[dm // 4]
                    nc.vector.tensor_copy(st[:, (dm % 4) * w:(dm % 4 + 1) * w],
                                          ps)
                    if dm == 3:
                        nc.sync.dma_start(
                            y_d[:, DKS * col0:DKS * col0 + 4 * w], stages[0])
                    elif dm == 7:
                        nc.sync.dma_start(
                            y_d[:, DKS * col0 + 4 * w:DKS * col0 + 8 * w],
                            stages[1])

    nc.compile()
    return nc


def _route_host(x, Wg, bg):
    """Reference-exact gate: fp64 logits, top-2 (ties -> lower index), softmax."""
    T = x.shape[0]
    logits = x.astype(np.float64) @ Wg.astype(np.float64) + bg.astype(np.float64)
    order = np.argsort(-logits, axis=1, kind="stable")[:, :TOP_K]  # [T, 2]
    vals = np.take_along_axis(logits, order, axis=1)
    vmax = vals.max(axis=1, keepdims=True)
    ev = np.exp(vals - vmax)
    w = (ev / ev.sum(axis=1, keepdims=True)).astype(np.float32)  # [T, 2]
    tok = np.repeat(np.arange(T), TOP_K)
    exp = order.ravel()
    wgt = w.ravel()
    tok_e, w_e = [], []
    for e in range(NUM_EXPERTS):
        m = exp == e
        tok_e.append(tok[m])
        w_e.append(wgt[m])
    return tok_e, w_e


def _pack_xs(xt, tok_e, tiles, Ctot):
    """Build the packed x stream [128, 8*Ctot] bf16 (identical per core).

    Column layout: expert-sorted pair stream, tile-major; within tile t of
    width w, dk block at cols [8*col0 + dk*w, ...)."""
    xT = np.zeros((MODEL_DIM, Ctot), dtype=BF16)
    col = 0
    seg = {}
    for e in range(NUM_EXPERTS):
        cnt = len(tok_e[e])
        if cnt == 0:
            continue
        xT[:, col:col + cnt] = xt[tok_e[e]].astype(BF16).T
        seg[e] = (col, cnt)
        col += sum(_tile_widths(cnt))
    out = np.empty((128, DKS * Ctot), dtype=BF16)
    for e, w, col0 in tiles:
        for dk in range(DKS):
            out[:, DKS * col0 + dk * w: DKS * col0 + (dk + 1) * w] = \
                xT[dk * 128:(dk + 1) * 128, col0:col0 + w]
    return np.ascontiguousarray(out), seg


def _pack_w1_slice(W1, k):
    """W1 [8, 1024, 4096] f32, core k -> [128, 8*4096] bf16.

    Block (e, fm, dk) at col e*4096 + (fm*8+dk)*128 =
    W1[e, dk*128:(dk+1)*128, k*512+fm*128 : +128]."""
    sl = W1[:, :, k * F_SLICE:(k + 1) * F_SLICE].astype(BF16)  # [8,1024,512]
    # [e, dk, 128, fm, 128] -> order (e, fm, dk)
    blk = sl.reshape(NUM_EXPERTS, DKS, 128, FMS, 128)
    out = blk.transpose(2, 0, 3, 1, 4).reshape(128, -1)
    return np.ascontiguousarray(out)


def _pack_w2_slice(W2, k):
    """W2 [8, 4096, 1024] f32, core k -> [128, 8*4096] bf16.

    Block (e, dm, fk) at col e*4096 + (dm*4+fk)*128 =
    W2[e, k*512+fk*128 : +128, dm*128:(dm+1)*128]."""
    sl = W2[:, k * F_SLICE:(k + 1) * F_SLICE, :].astype(BF16)  # [8,512,1024]
    blk = sl.reshape(NUM_EXPERTS, FMS, 128, DMS, 128)
    out = blk.transpose(2, 0, 3, 1, 4).reshape(128, -1)
    return np.ascontiguousarray(out)


def _pack_b1_slice(b1, k):
    """b1 [8, 4096] -> [128, 32] f32; col e*4+fm = b1[e, k*512+fm*128:+128]."""
    sl = b1[:, k * F_SLICE:(k + 1) * F_SLICE].astype(np.float32)  # [8, 512]
    out = sl.reshape(NUM_EXPERTS * FMS, 128).T
    return np.ascontiguousarray(out)


def _unpack_y(Yp, tiles, Ctot):
    """[128, 8*Ctot] tile-packed (dm-major inside tile) -> [Ctot, 1024]."""
    Y = np.empty((Ctot, MODEL_DIM), dtype=Yp.dtype)
    for e, w, col0 in tiles:
        for dm in range(DMS):
            Y[col0:col0 + w, dm * 128:(dm + 1) * 128] = \
                Yp[:, DKS * col0 + dm * w: DKS * col0 + (dm + 1) * w].T
    return Y


def _make_in_maps(xt, W1, b1, W2, tok_e, tiles, Ctot):
    xs_packed, seg = _pack_xs(xt, tok_e, tiles, Ctot)
    in_maps = []
    for k in range(N_CORES):
        in_maps.append({
            "w1": _pack_w1_slice(W1, k),
            "w2": _pack_w2_slice(W2, k),
            "b1": _pack_b1_slice(b1, k),
            "xs": xs_packed,
        })
    return in_maps, seg


def kernel(x, Wg, bg, W1, b1, W2, b2):
    from concourse.bass_utils import run_bass_kernel_spmd

    B, S, d = x.shape
    T = B * S
    xt = x.reshape(T, d)

    tok_e, w_e = _route_host(xt, Wg, bg)
    counts = [len(t) for t in tok_e]
    tiles, Ctot = _make_schedule(counts)

    if tiles not in _NC_CACHE:
        _NC_CACHE[tiles] = _build_tp_nc(tiles, Ctot)
    nc = _NC_CACHE[tiles]

    in_maps, seg = _make_in_maps(xt, W1, b1, W2, tok_e, tiles, Ctot)

    res = run_bass_kernel_spmd(nc, in_maps, core_ids=list(range(N_CORES)))

    # Combine: y(pair) = sum_k y_k(pair) + b2[e]; out[tok] += gate_w * y
    Yp = res.results[0]["y"].astype(np.float32)  # [128, 8*Ctot] packed
    for k in range(1, N_CORES):
        Yp += res.results[k]["y"].astype(np.float32)
    Y = _unpack_y(Yp, tiles, Ctot)               # [Ctot, MODEL_DIM]
    out = np.zeros((T, d), dtype=np.float32)
    for e in range(NUM_EXPERTS):
        if e not in seg:
            continue
        col0, cnt = seg[e]
        ye = Y[col0:col0 + cnt] + b2[e][None, :].astype(np.float32)
        out[tok_e[e]] += ye * w_e[e][:, None]
    return out.reshape(B, S, d)
